# revision 10
# baseline (speedup 1.0000x reference)
"""CenterHead (nms_detection) Trainium2 kernel.

Pipeline (reference.py semantics):
  y = relu(bn(conv3x3(x, sc_w) + sc_b))          # 256 -> 64
  heads = conv3x3(y, w_h) for 5 heads            # 64 -> 11 channels total
  hm = sigmoid(heads[0:3]); dims = exp(heads[6:9])
  per-class top-k(500), cross-class top-k, decode boxes

Device does the convs (the compute-heavy part) on 8 NeuronCores:
data-parallel over (sample, H-half) = 4 x 2 shards. Each core receives
its input rows with a 2-row halo (zero rows at image boundaries), runs
the shared conv + head convs in fp32 on the TensorEngine (col-tiled
pairs for the 64-channel shared conv, col-tiled quads for the 11-channel
heads), and writes the raw 11-channel head maps. Host applies
sigmoid/exp, top-k and box decode in fp32 replicating jax exactly.
"""
import json
import numpy as np

# ---------------------------------------------------------------------------
# problem geometry (hardcoded per contract)
B, CIN, H, W = 4, 256, 448, 448
COUT = 64              # shared conv output channels
NHEAD = 11             # hm(3) + center(2) + center_z(1) + dim(3) + rot(2)
HALF = 224             # rows per shard
XROWS = HALF + 4       # input rows provided per core (with halo+pad)
K = 500
PCR = np.array([0.0, -44.8, -2.0, 89.6, 44.8, 4.0], dtype=np.float32)
VOXEL = (0.2, 0.2)
STRIDE = 1.0
SCORE_THRESH = np.array([0.2, 0.3, 0.3], dtype=np.float32)
BN_EPS = 1e-5

NPAIR = HALF // 2      # 112 main-conv row pairs (y rows 0..223)
NYE = 6                # y_even ring slots
NYO = 6                # y_odd ring slots
NXR = 8                # x ring slots
WPAD = W + 2           # zero-padded row width


# ---------------------------------------------------------------------------
# BIR post-pass: this container's walrus accepts at most one sync wait per
# instruction; Tile attaches several (and ~10 on the tail Drain). Hoist all
# but one wait onto standalone EventSemaphore instructions inserted right
# before the offender on the same engine.
def _fix_multiwaits(json_bytes, max_waits=1):
    m = json.loads(json_bytes)
    for f in m.get("functions", []):
        for bb in f.get("blocks", []):
            out = []
            for ins in bb.get("instructions", []):
                si = ins.get("sync_info")
                waits = (si or {}).get("on_wait") or []
                if len(waits) > max_waits:
                    keep = waits[-max_waits:]
                    for i, w in enumerate(waits[: len(waits) - max_waits]):
                        out.append({
                            "debug": ins.get("debug"),
                            "engine": ins["engine"],
                            "ins": [],
                            "name": f"{ins.get('name', 'i')}__w{i}",
                            "opcode": "EventSemaphore",
                            "outs": [],
                            "sync_info": {"on_update": [], "on_wait": [w]},
                        })
                    si["on_wait"] = keep
                out.append(ins)
            bb["instructions"] = out
    return json.dumps(m).encode()


def _install_fixes():
    import concourse.bass as bass
    if getattr(bass.Bass, "_multiwait_fix_installed", False):
        return
    orig = bass.Bass.to_json_bytes

    def to_json_bytes(self):
        return _fix_multiwaits(orig(self))

    bass.Bass.to_json_bytes = to_json_bytes
    bass.Bass._multiwait_fix_installed = True


# ---------------------------------------------------------------------------
def _build_nc():
    import concourse.bass as bass
    import concourse.tile as tile
    from concourse import mybir

    f32 = mybir.dt.float32
    nc = bass.Bass(target_bir_lowering=False, trn_type="TRN2")

    bf16 = mybir.dt.float16
    xsh = nc.dram_tensor("xsh", [XROWS, CIN, W], bf16, kind="ExternalInput")
    xsl = nc.dram_tensor("xsl", [XROWS, CIN, W], bf16, kind="ExternalInput")
    wmh = nc.dram_tensor("wmh", [18, 128, COUT], bf16, kind="ExternalInput")
    wml = nc.dram_tensor("wml", [18, 128, COUT], bf16, kind="ExternalInput")
    wph = nc.dram_tensor("wph", [2, 3, 2, 128, 128], bf16, kind="ExternalInput")
    wpl = nc.dram_tensor("wpl", [2, 3, 2, 128, 128], bf16, kind="ExternalInput")
    wsh = nc.dram_tensor("wsh", [2, 3, 2, 128, COUT], bf16, kind="ExternalInput")
    wsl = nc.dram_tensor("wsl", [2, 3, 2, 128, COUT], bf16, kind="ExternalInput")
    wh128 = nc.dram_tensor("wh128", [3, 128, NHEAD], f32, kind="ExternalInput")
    wh64 = nc.dram_tensor("wh64", [3, 64, NHEAD], f32, kind="ExternalInput")
    bnc = nc.dram_tensor("bnc", [128, 1], f32, kind="ExternalInput")
    mlo = nc.dram_tensor("mlo", [128, 1], f32, kind="ExternalInput")
    mhi = nc.dram_tensor("mhi", [128, 1], f32, kind="ExternalInput")
    out = nc.dram_tensor("out", [HALF, NHEAD, W], f32, kind="ExternalOutput")

    xsh_r = xsh[:].rearrange("u (g p) w -> u p g w", p=128)
    xsl_r = xsl[:].rearrange("u (g p) w -> u p g w", p=128)
    relu = mybir.ActivationFunctionType.Relu

    with tile.TileContext(nc) as tc:
        with tc.tile_pool(name="fixed", bufs=1) as fixed, \
             tc.tile_pool(name="outp", bufs=3) as outp, \
             tc.tile_pool(name="psm", bufs=3, space="PSUM") as psm, \
             tc.tile_pool(name="psh", bufs=2, space="PSUM") as psh:

            # --- static tiles ------------------------------------------------
            wpht = fixed.tile([128, 2, 3, 2, 128], bf16)
            nc.gpsimd.dma_start(out=wpht[:],
                                in_=wph[:].rearrange("c d a k m -> k c d a m"))
            wplt = fixed.tile([128, 2, 3, 2, 128], bf16)
            nc.gpsimd.dma_start(out=wplt[:],
                                in_=wpl[:].rearrange("c d a k m -> k c d a m"))
            wsht = fixed.tile([128, 2, 3, 2, COUT], bf16)
            nc.gpsimd.dma_start(out=wsht[:],
                                in_=wsh[:].rearrange("c d s k m -> k c d s m"))
            wslt = fixed.tile([128, 2, 3, 2, COUT], bf16)
            nc.gpsimd.dma_start(out=wslt[:],
                                in_=wsl[:].rearrange("c d s k m -> k c d s m"))
            wmht = fixed.tile([128, 18, COUT], bf16)
            nc.gpsimd.dma_start(out=wmht[:], in_=wmh[:].rearrange("i k m -> k i m"))
            wmlt = fixed.tile([128, 18, COUT], bf16)
            nc.gpsimd.dma_start(out=wmlt[:], in_=wml[:].rearrange("i k m -> k i m"))
            wh128t = fixed.tile([128, 3, NHEAD], f32)
            nc.gpsimd.dma_start(out=wh128t[:],
                                in_=wh128[:].rearrange("d k m -> k d m"))
            # K=64 head weights live in partitions 64:128 so the matmul's
            # stationary and moving operands share a partition range.
            wh64t = fixed.tile([128, 3, NHEAD], f32)
            nc.vector.memset(wh64t[:], 0.0)
            nc.gpsimd.dma_start(out=wh64t[64:128, :, :],
                                in_=wh64[:].rearrange("d k m -> k d m"))
            bnct = fixed.tile([128, 1], f32)
            nc.gpsimd.dma_start(out=bnct[:], in_=bnc[:])
            mlot = fixed.tile([128, 1], f32)
            nc.gpsimd.dma_start(out=mlot[:], in_=mlo[:])
            mhit = fixed.tile([128, 1], f32)
            nc.gpsimd.dma_start(out=mhit[:], in_=mhi[:])

            # rings (persistent, manually indexed; pad columns stay zero)
            xrh = fixed.tile([128, NXR, 2, WPAD], bf16)
            nc.vector.memset(xrh[:], 0.0)
            xrl = fixed.tile([128, NXR, 2, WPAD], bf16)
            nc.vector.memset(xrl[:], 0.0)
            ye = fixed.tile([128, NYE, WPAD], f32)     # slot j: rows (2j, 2j+1)
            nc.vector.memset(ye[:], 0.0)
            yo = fixed.tile([128, NYO, WPAD], f32)     # slot j: rows (2j-1, 2j)
            nc.vector.memset(yo[:], 0.0)

            def load_x(u):
                """DMA input row u (u in -2..HALF+1) into its ring slots."""
                s = (u + 2) % NXR
                nc.sync.dma_start(out=xrh[:, s, :, 1:W + 1], in_=xsh_r[u + 2])
                nc.sync.dma_start(out=xrl[:, s, :, 1:W + 1], in_=xsl_r[u + 2])

            # 3-pass fp32-accurate bf16 conv: x*w ~ xh*wh + xl*wh + xh*wl
            PASSES = ((xrh, None), (xrl, None), (xrh, True))

            def xa(ring, u, kc, dx):
                s = (u + 2) % NXR
                return ring[:, s, kc, dx:dx + W]

            # preload x rows for row -1 and pairs 0..1
            for u in range(-2, 4):
                load_x(u)

            # --- shared conv helpers ----------------------------------------
            def conv_row_single(psum_half, row):
                """54 accumulating M=64 matmuls for one y row into psum[0:64]."""
                i = 0
                for kc in range(2):
                    for dy in (-1, 0, 1):
                        for dx in range(3):
                            wi = (kc * 3 + (dy + 1)) * 3 + dx
                            for ring, lo_w in PASSES:
                                wt = wmlt if lo_w else wmht
                                nc.tensor.matmul(
                                    psum_half, wt[:, wi, :],
                                    xa(ring, row + dy, kc, dx),
                                    start=(i == 0), stop=(i == 53),
                                    tile_position=(0, 0))
                                i += 1

            def conv_pair(pj):
                """y rows (2j, 2j+1) into one psum bank.

                Two dy-taps share each [128,128] stationary (lower half of M
                feeds out row 2j, upper half out row 2j+1), so moving row u
                carries taps dy=u-2j and dy=u-2j-1 in one matmul; the two
                edge rows (u=2j-1 / 2j+2) are single-tap M=64, col-tiled
                into one slot. 72 LDWEIGHTS per pair instead of 108."""
                pt = psm.tile([128, W], f32, tag="mainps")
                i = 0
                for kc in range(2):
                    for dx in range(3):
                        for ring, lo_w in PASSES:
                            wp = wplt if lo_w else wpht
                            ws = wslt if lo_w else wsht
                            st = (i == 0)
                            sp = (i == 17)
                            nc.tensor.matmul(
                                pt[:], wp[:, kc, dx, 0, :],
                                xa(ring, 2 * pj, kc, dx),
                                start=st, stop=False)
                            nc.tensor.matmul(
                                pt[:], wp[:, kc, dx, 1, :],
                                xa(ring, 2 * pj + 1, kc, dx),
                                start=False, stop=False)
                            nc.tensor.matmul(
                                pt[0:64], ws[:, kc, dx, 0, :],
                                xa(ring, 2 * pj - 1, kc, dx),
                                start=False, stop=sp, tile_position=(0, 0))
                            nc.tensor.matmul(
                                pt[64:128], ws[:, kc, dx, 1, :],
                                xa(ring, 2 * pj + 2, kc, dx),
                                start=False, stop=sp, tile_position=(0, 64))
                            i += 1
                return pt

            def bn_relu(dst, src, bias):
                nc.scalar.activation(out=dst, in_=src, func=relu,
                                     bias=bias, scale=1.0)

            # --- head conv ---------------------------------------------------
            def head_group(g):
                """Head rows 4g..4g+3 -> one psum bank (quad col groups),
                then DMA each row's [11, W] slice straight to DRAM."""
                pt = psh.tile([128, W], f32, tag="headps")
                for ci, (chunk, dx) in enumerate(
                        [(c, d) for c in (0, 1) for d in range(3)]):
                    st, sp = (ci == 0), (ci == 5)
                    for q in range(4):
                        t = 4 * g + q
                        if chunk == 0:   # K=128: y rows (t-1, t)
                            if t % 2 == 0:
                                rhs = yo[:, (t // 2) % NYO, dx:dx + W]
                            else:
                                rhs = ye[:, (t // 2) % NYE, dx:dx + W]
                            nc.tensor.matmul(
                                pt[32 * q:32 * q + NHEAD], wh128t[:, dx, :],
                                rhs, start=st, stop=sp,
                                tile_position=(0, 32 * q))
                        else:            # K=64: y row t+1 (upper half tiles)
                            if t % 2 == 0:
                                rhs = ye[64:128, (t // 2) % NYE, dx:dx + W]
                            else:
                                rhs = yo[64:128, ((t + 1) // 2) % NYO, dx:dx + W]
                            nc.tensor.matmul(
                                pt[32 * q:32 * q + NHEAD], wh64t[64:128, dx, :],
                                rhs, start=st, stop=sp,
                                tile_position=(64, 32 * q))
                ob = outp.tile([128, W], f32, tag="outsb")
                for q in range(4):
                    nc.vector.tensor_copy(ob[32 * q:32 * q + NHEAD, :],
                                          pt[32 * q:32 * q + NHEAD, :])
                for q in range(4):
                    t = 4 * g + q
                    nc.sync.dma_start(out=out[t],
                                      in_=ob[32 * q:32 * q + NHEAD, :])

            # --- y row -1 (lower half of yo[0]), zeroed on top-half cores ----
            pspec = psm.tile([128, W], f32, tag="mainps")
            conv_row_single(pspec[0:64], -1)
            bn_relu(yo[0:64, 0, 1:W + 1], pspec[0:64], bnct[0:64])
            nc.vector.tensor_scalar_mul(yo[0:64, 0, 1:W + 1],
                                        yo[0:64, 0, 1:W + 1], mlot[0:64])

            # --- main loop ---------------------------------------------------
            for j in range(NPAIR):
                for u in (2 * j + 3, 2 * j + 4):   # prefetch for pair j+1
                    if u <= HALF + 1:
                        load_x(u)
                pt = conv_pair(j)
                # y_even[j] <- rows (2j, 2j+1)
                bn_relu(ye[:, j % NYE, 1:W + 1], pt[:], bnct)
                # y_odd[j][64:] <- row 2j ; y_odd[j+1][0:64] <- row 2j+1
                bn_relu(yo[64:128, j % NYO, 1:W + 1], pt[0:64], bnct[64:128])
                bn_relu(yo[0:64, (j + 1) % NYO, 1:W + 1], pt[64:128],
                        bnct[0:64])
                # y row 224 (upper half of yo[112]); zeroed on bottom cores
                if j == NPAIR - 1:
                    pf = psm.tile([128, W], f32, tag="mainps")
                    conv_row_single(pf[0:64], HALF)
                    bn_relu(yo[64:128, NPAIR % NYO, 1:W + 1], pf[0:64],
                            bnct[64:128])
                    nc.vector.tensor_scalar_mul(
                        yo[64:128, NPAIR % NYO, 1:W + 1],
                        yo[64:128, NPAIR % NYO, 1:W + 1], mhit[64:128])
                # head group g is ready once pair 2g+2 is done
                if j >= 2 and j % 2 == 0:
                    head_group((j - 2) // 2)
            head_group(NPAIR // 2 - 1)             # g = 55 (rows 220..223)
    return nc


# ---------------------------------------------------------------------------
def _pack_weights(sc_w, sc_b, bn_gamma, bn_beta, bn_mean, bn_var,
                  hm_w, hm_b, center_w, center_b, centerz_w, centerz_b,
                  dim_w, dim_b, rot_w, rot_b):
    k = (bn_gamma / np.sqrt(bn_var + BN_EPS)).astype(np.float32)   # [64]
    c = ((sc_b - bn_mean) * k + bn_beta).astype(np.float32)        # [64]

    bf16 = np.float16
    # shared conv weights, BN scale folded in: wm[i][kk, m]
    wm = np.empty((18, 128, COUT), np.float32)
    for kc in range(2):
        for dyi in range(3):
            for dx in range(3):
                i = (kc * 3 + dyi) * 3 + dx
                blk = sc_w[:, kc * 128:(kc + 1) * 128, dyi, dx]    # [64,128]
                wm[i] = (blk * k[:, None]).T.astype(np.float32)
    wm_hi = wm.astype(bf16)
    wm_lo = (wm - wm_hi.astype(np.float32)).astype(bf16)

    def tap(kc, dyi, dx):
        return wm[(kc * 3 + dyi) * 3 + dx]          # [128, 64] fp32

    wp = np.zeros((2, 3, 2, 128, 128), np.float32)
    ws = np.zeros((2, 3, 2, 128, COUT), np.float32)
    for kc in range(2):
        for dx in range(3):
            # A (moving row 2j): lower<-dy=0 (idx1), upper<-dy=-1 (idx0)
            wp[kc, dx, 0, :, 0:64] = tap(kc, 1, dx)
            wp[kc, dx, 0, :, 64:128] = tap(kc, 0, dx)
            # B (moving row 2j+1): lower<-dy=+1 (idx2), upper<-dy=0 (idx1)
            wp[kc, dx, 1, :, 0:64] = tap(kc, 2, dx)
            wp[kc, dx, 1, :, 64:128] = tap(kc, 1, dx)
            ws[kc, dx, 0] = tap(kc, 0, dx)           # row 2j-1 -> lower, dy=-1
            ws[kc, dx, 1] = tap(kc, 2, dx)           # row 2j+2 -> upper, dy=+1
    wp_hi = wp.astype(bf16)
    wp_lo = (wp - wp_hi.astype(np.float32)).astype(bf16)
    ws_hi = ws.astype(bf16)
    ws_lo = (ws - ws_hi.astype(np.float32)).astype(bf16)

    hw = np.concatenate([hm_w, center_w, centerz_w, dim_w, rot_w], axis=0)
    hb = np.concatenate([hm_b, center_b, centerz_b, dim_b, rot_b], axis=0)
    # wh128[dx][kk, m]: kk<64 -> y[t-1] (dy idx 0); kk>=64 -> y[t] (dy idx 1)
    wh128 = np.empty((3, 128, NHEAD), np.float32)
    wh64 = np.empty((3, 64, NHEAD), np.float32)
    for dx in range(3):
        wh128[dx, 0:64] = hw[:, :, 0, dx].T
        wh128[dx, 64:128] = hw[:, :, 1, dx].T
        wh64[dx] = hw[:, :, 2, dx].T

    bnc2 = np.concatenate([c, c]).reshape(128, 1).astype(np.float32)
    return ((np.ascontiguousarray(wm_hi), np.ascontiguousarray(wm_lo),
             np.ascontiguousarray(wp_hi), np.ascontiguousarray(wp_lo),
             np.ascontiguousarray(ws_hi), np.ascontiguousarray(ws_lo)),
            np.ascontiguousarray(wh128),
            np.ascontiguousarray(wh64), bnc2, hb.astype(np.float32))


def _run_device(x, wm, wh128, wh64, bnc2, trace=False):
    _install_fixes()
    from concourse.bass_utils import run_bass_kernel_spmd
    bf16 = np.float16

    wm_hi, wm_lo, wp_hi, wp_lo, ws_hi, ws_lo = wm
    nc = _build_nc()
    ones = np.ones((128, 1), np.float32)
    zeros = np.zeros((128, 1), np.float32)
    in_maps = []
    for b in range(B):
        for half in range(2):
            xs = np.zeros((XROWS, CIN, W), np.float32)
            if half == 0:
                # local row u in -2..225 <-> global u; rows -2,-1 zero-pad
                xs[2:XROWS] = x[b, :, 0:HALF + 2].transpose(1, 0, 2)
                mlo, mhi = zeros, ones     # y[-1] is image padding -> zero
            else:
                # local u <-> global 224+u; real global rows 222..447
                xs[0:HALF + 2] = x[b, :, HALF - 2:H].transpose(1, 0, 2)
                mlo, mhi = ones, zeros     # y[224] is image padding -> zero
            xs_hi = xs.astype(bf16)
            xs_lo = (xs - xs_hi.astype(np.float32)).astype(bf16)
            in_maps.append({
                "xsh": np.ascontiguousarray(xs_hi),
                "xsl": np.ascontiguousarray(xs_lo),
                "wmh": wm_hi, "wml": wm_lo,
                "wph": wp_hi, "wpl": wp_lo, "wsh": ws_hi, "wsl": ws_lo,
                "wh128": wh128, "wh64": wh64, "bnc": bnc2,
                "mlo": mlo, "mhi": mhi,
            })
    res = run_bass_kernel_spmd(nc, in_maps, core_ids=list(range(8)),
                               trace=trace)
    maps = np.empty((B, NHEAD, H, W), np.float32)
    for b in range(B):
        for half in range(2):
            o = res.results[2 * b + half]["out"]     # [HALF, NHEAD, W]
            maps[b, :, half * HALF:(half + 1) * HALF] = o.transpose(1, 0, 2)
    return maps, res


# ---------------------------------------------------------------------------
def _decode(maps, hb):
    """Replicate reference.decode exactly with jax fp32 ops on host."""
    import jax
    with jax.default_device(jax.devices("cpu")[0]):
        return _decode_impl(maps, hb)


def _decode_impl(maps, hb):
    import jax
    import jax.numpy as jnp

    maps = jnp.asarray(maps) + jnp.asarray(hb)[None, :, None, None]
    hm = jax.nn.sigmoid(maps[:, 0:3])
    center = maps[:, 3:5]
    center_z = maps[:, 5:6]
    dim3 = jnp.exp(maps[:, 6:9])
    rot = maps[:, 9:11]

    C = 3
    pcr = jnp.asarray(PCR)
    thr = jnp.asarray(SCORE_THRESH)
    s1, i1 = jax.lax.top_k(hm.reshape(B, C, H * W), K)          # [B,C,K]
    ys1 = (i1 // W).astype(jnp.float32)
    xs1 = (i1 % W).astype(jnp.float32)
    scores, i2 = jax.lax.top_k(s1.reshape(B, C * K), K)          # [B,K]
    class_ids = (i2 // K).astype(jnp.int32)
    inds = jnp.take_along_axis(i1.reshape(B, C * K), i2, axis=1)
    ys = jnp.take_along_axis(ys1.reshape(B, C * K), i2, axis=1)
    xs = jnp.take_along_axis(xs1.reshape(B, C * K), i2, axis=1)

    def gather(feat):
        D = feat.shape[1]
        f = feat.transpose(0, 2, 3, 1).reshape(B, H * W, D)
        return jnp.take_along_axis(f, inds[:, :, None], axis=1)

    ctr = gather(center)
    cz = gather(center_z)
    d3 = gather(dim3)
    r = gather(rot)
    angle = jnp.arctan2(r[:, :, 1:2], r[:, :, 0:1])
    xs = (xs[:, :, None] + ctr[:, :, 0:1]) * STRIDE * VOXEL[0] + pcr[0]
    ys = (ys[:, :, None] + ctr[:, :, 1:2]) * STRIDE * VOXEL[1] + pcr[1]
    boxes = jnp.concatenate([xs, ys, cz, d3, angle], axis=-1)
    mask = jnp.all(boxes[..., :3] >= pcr[:3], axis=-1)
    mask &= jnp.all(boxes[..., :3] <= pcr[3:], axis=-1)
    mask &= scores > thr[class_ids]
    return (np.asarray(boxes), np.asarray(scores),
            np.asarray(class_ids), np.asarray(mask))


def kernel(x, sc_w, sc_b, bn_gamma, bn_beta, bn_mean, bn_var,
           hm_w, hm_b, center_w, center_b, centerz_w, centerz_b,
           dim_w, dim_b, rot_w, rot_b, _trace=False, _ret_res=False):
    args = [np.asarray(a, np.float32) for a in
            (sc_w, sc_b, bn_gamma, bn_beta, bn_mean, bn_var, hm_w, hm_b,
             center_w, center_b, centerz_w, centerz_b, dim_w, dim_b,
             rot_w, rot_b)]
    x = np.asarray(x, np.float32)
    wm, wh128, wh64, bnc2, hb = _pack_weights(*args)
    maps, res = _run_device(x, wm, wh128, wh64, bnc2, trace=_trace)
    outs = _decode(maps, hb)
    if _ret_res:
        return outs, res, maps
    return outs


# revision 11
# speedup vs baseline: 1.3057x; 1.3057x over previous
"""CenterHead (nms_detection) Trainium2 kernel.

Pipeline (reference.py semantics):
  y = relu(bn(conv3x3(x, sc_w) + sc_b))          # 256 -> 64
  heads = conv3x3(y, w_h) for 5 heads            # 64 -> 11 channels total
  hm = sigmoid(heads[0:3]); dims = exp(heads[6:9])
  per-class top-k(500), cross-class top-k, decode boxes

Device does the convs (the compute-heavy part) on 8 NeuronCores:
data-parallel over (sample, H-half) = 4 x 2 shards. Each core receives
its input rows with a 2-row halo (zero rows at image boundaries), runs
the shared conv + head convs in fp32 on the TensorEngine (col-tiled
pairs for the 64-channel shared conv, col-tiled quads for the 11-channel
heads), and writes the raw 11-channel head maps. Host applies
sigmoid/exp, top-k and box decode in fp32 replicating jax exactly.
"""
import json
import numpy as np

# ---------------------------------------------------------------------------
# problem geometry (hardcoded per contract)
B, CIN, H, W = 4, 256, 448, 448
COUT = 64              # shared conv output channels
NHEAD = 11             # hm(3) + center(2) + center_z(1) + dim(3) + rot(2)
HALF = 224             # rows per shard
XROWS = HALF + 4       # input rows provided per core (with halo+pad)
K = 500
PCR = np.array([0.0, -44.8, -2.0, 89.6, 44.8, 4.0], dtype=np.float32)
VOXEL = (0.2, 0.2)
STRIDE = 1.0
SCORE_THRESH = np.array([0.2, 0.3, 0.3], dtype=np.float32)
BN_EPS = 1e-5

NPAIR = HALF // 2      # 112 main-conv row pairs (y rows 0..223)
NYE = 6                # y_even ring slots
NYO = 6                # y_odd ring slots
NXR = 8                # x ring slots
WPAD = W + 2           # zero-padded row width


# ---------------------------------------------------------------------------
# BIR post-pass: this container's walrus accepts at most one sync wait per
# instruction; Tile attaches several (and ~10 on the tail Drain). Hoist all
# but one wait onto standalone EventSemaphore instructions inserted right
# before the offender on the same engine.
def _fix_multiwaits(json_bytes, max_waits=1):
    m = json.loads(json_bytes)
    for f in m.get("functions", []):
        for bb in f.get("blocks", []):
            out = []
            for ins in bb.get("instructions", []):
                si = ins.get("sync_info")
                waits = (si or {}).get("on_wait") or []
                if len(waits) > max_waits:
                    keep = waits[-max_waits:]
                    for i, w in enumerate(waits[: len(waits) - max_waits]):
                        out.append({
                            "debug": ins.get("debug"),
                            "engine": ins["engine"],
                            "ins": [],
                            "name": f"{ins.get('name', 'i')}__w{i}",
                            "opcode": "EventSemaphore",
                            "outs": [],
                            "sync_info": {"on_update": [], "on_wait": [w]},
                        })
                    si["on_wait"] = keep
                out.append(ins)
            bb["instructions"] = out
    return json.dumps(m).encode()


def _install_fixes():
    import concourse.bass as bass
    if getattr(bass.Bass, "_multiwait_fix_installed", False):
        return
    orig = bass.Bass.to_json_bytes

    def to_json_bytes(self):
        return _fix_multiwaits(orig(self))

    bass.Bass.to_json_bytes = to_json_bytes
    bass.Bass._multiwait_fix_installed = True


# ---------------------------------------------------------------------------
def _build_nc():
    import concourse.bass as bass
    import concourse.tile as tile
    from concourse import mybir

    f32 = mybir.dt.float32
    nc = bass.Bass(target_bir_lowering=False, trn_type="TRN2")

    bf16 = mybir.dt.float16
    xsh = nc.dram_tensor("xsh", [XROWS, CIN, W], bf16, kind="ExternalInput")
    xsl = nc.dram_tensor("xsl", [XROWS, CIN, W], bf16, kind="ExternalInput")
    wmh = nc.dram_tensor("wmh", [18, 128, COUT], bf16, kind="ExternalInput")
    wml = nc.dram_tensor("wml", [18, 128, COUT], bf16, kind="ExternalInput")
    wh128 = nc.dram_tensor("wh128", [3, 128, NHEAD], f32, kind="ExternalInput")
    wh64 = nc.dram_tensor("wh64", [3, 64, NHEAD], f32, kind="ExternalInput")
    bnc = nc.dram_tensor("bnc", [128, 1], f32, kind="ExternalInput")
    mlo = nc.dram_tensor("mlo", [128, 1], f32, kind="ExternalInput")
    mhi = nc.dram_tensor("mhi", [128, 1], f32, kind="ExternalInput")
    out = nc.dram_tensor("out", [HALF, NHEAD, W], f32, kind="ExternalOutput")

    xsh_r = xsh[:].rearrange("u (g p) w -> u p g w", p=128)
    xsl_r = xsl[:].rearrange("u (g p) w -> u p g w", p=128)
    relu = mybir.ActivationFunctionType.Relu

    with tile.TileContext(nc) as tc:
        with tc.tile_pool(name="fixed", bufs=1) as fixed, \
             tc.tile_pool(name="outp", bufs=3) as outp, \
             tc.tile_pool(name="psm", bufs=3, space="PSUM") as psm, \
             tc.tile_pool(name="psh", bufs=2, space="PSUM") as psh:

            # --- static tiles ------------------------------------------------
            wmht = fixed.tile([128, 18, COUT], bf16)
            nc.gpsimd.dma_start(out=wmht[:], in_=wmh[:].rearrange("i k m -> k i m"))
            wmlt = fixed.tile([128, 18, COUT], bf16)
            nc.gpsimd.dma_start(out=wmlt[:], in_=wml[:].rearrange("i k m -> k i m"))
            wh128t = fixed.tile([128, 3, NHEAD], f32)
            nc.gpsimd.dma_start(out=wh128t[:],
                                in_=wh128[:].rearrange("d k m -> k d m"))
            # K=64 head weights live in partitions 64:128 so the matmul's
            # stationary and moving operands share a partition range.
            wh64t = fixed.tile([128, 3, NHEAD], f32)
            nc.vector.memset(wh64t[:], 0.0)
            nc.gpsimd.dma_start(out=wh64t[64:128, :, :],
                                in_=wh64[:].rearrange("d k m -> k d m"))
            bnct = fixed.tile([128, 1], f32)
            nc.gpsimd.dma_start(out=bnct[:], in_=bnc[:])
            mlot = fixed.tile([128, 1], f32)
            nc.gpsimd.dma_start(out=mlot[:], in_=mlo[:])
            mhit = fixed.tile([128, 1], f32)
            nc.gpsimd.dma_start(out=mhit[:], in_=mhi[:])

            # rings (persistent, manually indexed; pad columns stay zero)
            xrh = fixed.tile([128, NXR, 2, WPAD], bf16)
            nc.vector.memset(xrh[:], 0.0)
            xrl = fixed.tile([128, NXR, 2, WPAD], bf16)
            nc.vector.memset(xrl[:], 0.0)
            ye = fixed.tile([128, NYE, WPAD], f32)     # slot j: rows (2j, 2j+1)
            nc.vector.memset(ye[:], 0.0)
            yo = fixed.tile([128, NYO, WPAD], f32)     # slot j: rows (2j-1, 2j)
            nc.vector.memset(yo[:], 0.0)

            def load_x(u):
                """DMA input row u (u in -2..HALF+1) into its ring slots."""
                s = (u + 2) % NXR
                nc.sync.dma_start(out=xrh[:, s, :, 1:W + 1], in_=xsh_r[u + 2])
                nc.sync.dma_start(out=xrl[:, s, :, 1:W + 1], in_=xsl_r[u + 2])

            # 3-pass fp32-accurate bf16 conv: x*w ~ xh*wh + xl*wh + xh*wl
            PASSES = ((xrh, None), (xrl, None), (xrh, True))

            def xa(ring, u, kc, dx):
                s = (u + 2) % NXR
                return ring[:, s, kc, dx:dx + W]

            # preload x rows for row -1 and pairs 0..1
            for u in range(-2, 4):
                load_x(u)

            # --- shared conv helpers ----------------------------------------
            def conv_row_single(psum_half, row):
                """54 accumulating M=64 matmuls for one y row into psum[0:64]."""
                i = 0
                for kc in range(2):
                    for dy in (-1, 0, 1):
                        for dx in range(3):
                            wi = (kc * 3 + (dy + 1)) * 3 + dx
                            for ring, lo_w in PASSES:
                                wt = wmlt if lo_w else wmht
                                nc.tensor.matmul(
                                    psum_half, wt[:, wi, :],
                                    xa(ring, row + dy, kc, dx),
                                    start=(i == 0), stop=(i == 53),
                                    tile_position=(0, 0))
                                i += 1

            def conv_pair(pj):
                """y rows (2j, 2j+1) into one psum bank, col-tiled halves."""
                pt = psm.tile([128, W], f32, tag="mainps")
                i = 0
                for kc in range(2):
                    for dy in (-1, 0, 1):
                        for dx in range(3):
                            wi = (kc * 3 + (dy + 1)) * 3 + dx
                            for ring, lo_w in PASSES:
                                wt = wmlt if lo_w else wmht
                                st, sp = (i == 0), (i == 53)
                                nc.tensor.matmul(
                                    pt[0:64], wt[:, wi, :],
                                    xa(ring, 2 * pj + dy, kc, dx),
                                    start=st, stop=sp, tile_position=(0, 0))
                                nc.tensor.matmul(
                                    pt[64:128], wt[:, wi, :],
                                    xa(ring, 2 * pj + 1 + dy, kc, dx),
                                    start=st, stop=sp, tile_position=(0, 64))
                                i += 1
                return pt

            def bn_relu(dst, src, bias):
                nc.scalar.activation(out=dst, in_=src, func=relu,
                                     bias=bias, scale=1.0)

            # --- head conv ---------------------------------------------------
            def head_group(g):
                """Head rows 4g..4g+3 -> one psum bank (quad col groups),
                then DMA each row's [11, W] slice straight to DRAM."""
                pt = psh.tile([128, W], f32, tag="headps")
                for ci, (chunk, dx) in enumerate(
                        [(c, d) for c in (0, 1) for d in range(3)]):
                    st, sp = (ci == 0), (ci == 5)
                    for q in range(4):
                        t = 4 * g + q
                        if chunk == 0:   # K=128: y rows (t-1, t)
                            if t % 2 == 0:
                                rhs = yo[:, (t // 2) % NYO, dx:dx + W]
                            else:
                                rhs = ye[:, (t // 2) % NYE, dx:dx + W]
                            nc.tensor.matmul(
                                pt[32 * q:32 * q + NHEAD], wh128t[:, dx, :],
                                rhs, start=st, stop=sp,
                                tile_position=(0, 32 * q))
                        else:            # K=64: y row t+1 (upper half tiles)
                            if t % 2 == 0:
                                rhs = ye[64:128, (t // 2) % NYE, dx:dx + W]
                            else:
                                rhs = yo[64:128, ((t + 1) // 2) % NYO, dx:dx + W]
                            nc.tensor.matmul(
                                pt[32 * q:32 * q + NHEAD], wh64t[64:128, dx, :],
                                rhs, start=st, stop=sp,
                                tile_position=(64, 32 * q))
                ob = outp.tile([128, W], f32, tag="outsb")
                for q in range(4):
                    nc.vector.tensor_copy(ob[32 * q:32 * q + NHEAD, :],
                                          pt[32 * q:32 * q + NHEAD, :])
                for q in range(4):
                    t = 4 * g + q
                    nc.sync.dma_start(out=out[t],
                                      in_=ob[32 * q:32 * q + NHEAD, :])

            # --- y row -1 (lower half of yo[0]), zeroed on top-half cores ----
            pspec = psm.tile([128, W], f32, tag="mainps")
            conv_row_single(pspec[0:64], -1)
            bn_relu(yo[0:64, 0, 1:W + 1], pspec[0:64], bnct[0:64])
            nc.vector.tensor_scalar_mul(yo[0:64, 0, 1:W + 1],
                                        yo[0:64, 0, 1:W + 1], mlot[0:64])

            # --- main loop ---------------------------------------------------
            for j in range(NPAIR):
                for u in (2 * j + 3, 2 * j + 4):   # prefetch for pair j+1
                    if u <= HALF + 1:
                        load_x(u)
                pt = conv_pair(j)
                # y_even[j] <- rows (2j, 2j+1)
                bn_relu(ye[:, j % NYE, 1:W + 1], pt[:], bnct)
                # y_odd[j][64:] <- row 2j ; y_odd[j+1][0:64] <- row 2j+1
                bn_relu(yo[64:128, j % NYO, 1:W + 1], pt[0:64], bnct[64:128])
                bn_relu(yo[0:64, (j + 1) % NYO, 1:W + 1], pt[64:128],
                        bnct[0:64])
                # y row 224 (upper half of yo[112]); zeroed on bottom cores
                if j == NPAIR - 1:
                    pf = psm.tile([128, W], f32, tag="mainps")
                    conv_row_single(pf[0:64], HALF)
                    bn_relu(yo[64:128, NPAIR % NYO, 1:W + 1], pf[0:64],
                            bnct[64:128])
                    nc.vector.tensor_scalar_mul(
                        yo[64:128, NPAIR % NYO, 1:W + 1],
                        yo[64:128, NPAIR % NYO, 1:W + 1], mhit[64:128])
                # head group g is ready once pair 2g+2 is done
                if j >= 2 and j % 2 == 0:
                    head_group((j - 2) // 2)
            head_group(NPAIR // 2 - 1)             # g = 55 (rows 220..223)
    return nc


# ---------------------------------------------------------------------------
def _pack_weights(sc_w, sc_b, bn_gamma, bn_beta, bn_mean, bn_var,
                  hm_w, hm_b, center_w, center_b, centerz_w, centerz_b,
                  dim_w, dim_b, rot_w, rot_b):
    k = (bn_gamma / np.sqrt(bn_var + BN_EPS)).astype(np.float32)   # [64]
    c = ((sc_b - bn_mean) * k + bn_beta).astype(np.float32)        # [64]

    bf16 = np.float16
    # shared conv weights, BN scale folded in: wm[i][kk, m]
    wm = np.empty((18, 128, COUT), np.float32)
    for kc in range(2):
        for dyi in range(3):
            for dx in range(3):
                i = (kc * 3 + dyi) * 3 + dx
                blk = sc_w[:, kc * 128:(kc + 1) * 128, dyi, dx]    # [64,128]
                wm[i] = (blk * k[:, None]).T.astype(np.float32)
    wm_hi = wm.astype(bf16)
    wm_lo = (wm - wm_hi.astype(np.float32)).astype(bf16)

    hw = np.concatenate([hm_w, center_w, centerz_w, dim_w, rot_w], axis=0)
    hb = np.concatenate([hm_b, center_b, centerz_b, dim_b, rot_b], axis=0)
    # wh128[dx][kk, m]: kk<64 -> y[t-1] (dy idx 0); kk>=64 -> y[t] (dy idx 1)
    wh128 = np.empty((3, 128, NHEAD), np.float32)
    wh64 = np.empty((3, 64, NHEAD), np.float32)
    for dx in range(3):
        wh128[dx, 0:64] = hw[:, :, 0, dx].T
        wh128[dx, 64:128] = hw[:, :, 1, dx].T
        wh64[dx] = hw[:, :, 2, dx].T

    bnc2 = np.concatenate([c, c]).reshape(128, 1).astype(np.float32)
    return ((np.ascontiguousarray(wm_hi), np.ascontiguousarray(wm_lo)),
            np.ascontiguousarray(wh128),
            np.ascontiguousarray(wh64), bnc2, hb.astype(np.float32))


def _run_device(x, wm, wh128, wh64, bnc2, trace=False):
    _install_fixes()
    from concourse.bass_utils import run_bass_kernel_spmd
    bf16 = np.float16

    wm_hi, wm_lo = wm
    nc = _build_nc()
    ones = np.ones((128, 1), np.float32)
    zeros = np.zeros((128, 1), np.float32)
    in_maps = []
    for b in range(B):
        for half in range(2):
            xs = np.zeros((XROWS, CIN, W), np.float32)
            if half == 0:
                # local row u in -2..225 <-> global u; rows -2,-1 zero-pad
                xs[2:XROWS] = x[b, :, 0:HALF + 2].transpose(1, 0, 2)
                mlo, mhi = zeros, ones     # y[-1] is image padding -> zero
            else:
                # local u <-> global 224+u; real global rows 222..447
                xs[0:HALF + 2] = x[b, :, HALF - 2:H].transpose(1, 0, 2)
                mlo, mhi = ones, zeros     # y[224] is image padding -> zero
            xs_hi = xs.astype(bf16)
            xs_lo = (xs - xs_hi.astype(np.float32)).astype(bf16)
            in_maps.append({
                "xsh": np.ascontiguousarray(xs_hi),
                "xsl": np.ascontiguousarray(xs_lo),
                "wmh": wm_hi, "wml": wm_lo,
                "wh128": wh128, "wh64": wh64, "bnc": bnc2,
                "mlo": mlo, "mhi": mhi,
            })
    res = run_bass_kernel_spmd(nc, in_maps, core_ids=list(range(8)),
                               trace=trace)
    maps = np.empty((B, NHEAD, H, W), np.float32)
    for b in range(B):
        for half in range(2):
            o = res.results[2 * b + half]["out"]     # [HALF, NHEAD, W]
            maps[b, :, half * HALF:(half + 1) * HALF] = o.transpose(1, 0, 2)
    return maps, res


# ---------------------------------------------------------------------------
def _decode(maps, hb):
    """Replicate reference.decode exactly with jax fp32 ops on host."""
    import jax
    with jax.default_device(jax.devices("cpu")[0]):
        return _decode_impl(maps, hb)


def _decode_impl(maps, hb):
    import jax
    import jax.numpy as jnp

    maps = jnp.asarray(maps) + jnp.asarray(hb)[None, :, None, None]
    hm = jax.nn.sigmoid(maps[:, 0:3])
    center = maps[:, 3:5]
    center_z = maps[:, 5:6]
    dim3 = jnp.exp(maps[:, 6:9])
    rot = maps[:, 9:11]

    C = 3
    pcr = jnp.asarray(PCR)
    thr = jnp.asarray(SCORE_THRESH)
    s1, i1 = jax.lax.top_k(hm.reshape(B, C, H * W), K)          # [B,C,K]
    ys1 = (i1 // W).astype(jnp.float32)
    xs1 = (i1 % W).astype(jnp.float32)
    scores, i2 = jax.lax.top_k(s1.reshape(B, C * K), K)          # [B,K]
    class_ids = (i2 // K).astype(jnp.int32)
    inds = jnp.take_along_axis(i1.reshape(B, C * K), i2, axis=1)
    ys = jnp.take_along_axis(ys1.reshape(B, C * K), i2, axis=1)
    xs = jnp.take_along_axis(xs1.reshape(B, C * K), i2, axis=1)

    def gather(feat):
        D = feat.shape[1]
        f = feat.transpose(0, 2, 3, 1).reshape(B, H * W, D)
        return jnp.take_along_axis(f, inds[:, :, None], axis=1)

    ctr = gather(center)
    cz = gather(center_z)
    d3 = gather(dim3)
    r = gather(rot)
    angle = jnp.arctan2(r[:, :, 1:2], r[:, :, 0:1])
    xs = (xs[:, :, None] + ctr[:, :, 0:1]) * STRIDE * VOXEL[0] + pcr[0]
    ys = (ys[:, :, None] + ctr[:, :, 1:2]) * STRIDE * VOXEL[1] + pcr[1]
    boxes = jnp.concatenate([xs, ys, cz, d3, angle], axis=-1)
    mask = jnp.all(boxes[..., :3] >= pcr[:3], axis=-1)
    mask &= jnp.all(boxes[..., :3] <= pcr[3:], axis=-1)
    mask &= scores > thr[class_ids]
    return (np.asarray(boxes), np.asarray(scores),
            np.asarray(class_ids), np.asarray(mask))


def kernel(x, sc_w, sc_b, bn_gamma, bn_beta, bn_mean, bn_var,
           hm_w, hm_b, center_w, center_b, centerz_w, centerz_b,
           dim_w, dim_b, rot_w, rot_b, _trace=False, _ret_res=False):
    args = [np.asarray(a, np.float32) for a in
            (sc_w, sc_b, bn_gamma, bn_beta, bn_mean, bn_var, hm_w, hm_b,
             center_w, center_b, centerz_w, centerz_b, dim_w, dim_b,
             rot_w, rot_b)]
    x = np.asarray(x, np.float32)
    wm, wh128, wh64, bnc2, hb = _pack_weights(*args)
    maps, res = _run_device(x, wm, wh128, wh64, bnc2, trace=_trace)
    outs = _decode(maps, hb)
    if _ret_res:
        return outs, res, maps
    return outs
